# revision 82
# baseline (speedup 1.0000x reference)
"""Distributed multi-head attention kernel for 8 TRN2 NeuronCores.

Sharding: data parallel over batch (B=2 -> 2 groups of 4 cores), tensor
parallel over heads within each group (16 heads -> 4 heads/core).
Each core computes q/k/v projections for its 4 heads, rope, causal
attention, and a partial out-projection through its 256 columns of wo.
The host casts inputs to bf16, sums the 4 partial outputs per batch in
f32, and assembles k/v.

All matmuls run in bf16 (f32 PSUM accumulation). Attention is computed
in "transposed" layout (channels on partitions, time on the free dim)
so that no on-chip transposes of the attention matrix are needed:
    S.T[k,q]  = kT.T @ qT          (lhsT=kT chunk, rhs=qT)
    outT[d,q] = [v|1].T @ expS.T   (lhsT=v natural + ones col, rhs=expS.T)
The ones column accumulates the softmax denominator for free.
"""

import numpy as np
import ml_dtypes

import concourse.bass as bass
import concourse.tile as tile
from concourse import bacc, mybir
from concourse.bass_utils import run_bass_kernel_spmd

F32 = mybir.dt.float32
BF16 = mybir.dt.bfloat16
BF16NP = ml_dtypes.bfloat16

B, T, C = 2, 2048, 1024
H, HD = 16, 64
NCORES = 8
CPB = NCORES // B        # cores per batch = 4
HPC = H // CPB           # heads per core = 4
HL = HPC * HD            # local channels = 256
MCH = HL // 128          # 128-partition chunks of local channels = 2
CCH = C // 128           # 128-chunks of model dim = 8
TCH = T // 128           # 128-chunks of time = 16
NT = 512                 # PSUM bank tile (f32)
ST = 1024                # attention strip width

_CACHE = {}


def _rope_tables():
    # Matches reference _rope (f32 math), then bf16 for 4x DVE ops.
    # Replicated to 128 partitions so every 32-row slice is aligned.
    inv_freq = (1.0 / (10000.0 ** (np.arange(0, HD, 2, dtype=np.float32) / HD))).astype(np.float32)
    pos = np.arange(T, dtype=np.float32)
    freqs = pos[None, :] * inv_freq[:, None]          # [32, T]
    cos = np.tile(np.cos(freqs), (4, 1)).astype(BF16NP)
    # Sign-folded sin table for the pre-swapped product trick:
    # rows [+sin, -sin, +sin, -sin] by 32-row groups.
    s = np.sin(freqs)
    sin2 = np.concatenate([s, -s, s, -s], axis=0).astype(BF16NP)
    return cos, sin2


def _build():
    nc = bacc.Bacc(
        "TRN2",
        target_bir_lowering=False,
        debug=False,
        enable_asserts=False,
        num_devices=NCORES,
    )

    x_e = nc.dram_tensor("x", [T, C], BF16, kind="ExternalInput").ap()
    wq_e = nc.dram_tensor("wq", [HL, C], BF16, kind="ExternalInput").ap()
    wk_e = nc.dram_tensor("wk", [HL, C], BF16, kind="ExternalInput").ap()
    wv_e = nc.dram_tensor("wv", [HL, C], BF16, kind="ExternalInput").ap()
    wo_e = nc.dram_tensor("wo", [C, HL], BF16, kind="ExternalInput").ap()

    outp = nc.dram_tensor("outp", [T, C], BF16, kind="ExternalOutput").ap()
    kt_o = nc.dram_tensor("kt", [HL, T], BF16, kind="ExternalOutput").ap()
    vt_o = nc.dram_tensor("vo", [T, HL], BF16, kind="ExternalOutput").ap()

    cos_np, sin_np = _rope_tables()
    cos_d = nc.inline_tensor(cos_np, name="cos_tab").ap()
    sin_d = nc.inline_tensor(sin_np, name="sin_tab").ap()
    # Causal keep-mask in (k, q) layout: keep where k <= q.
    tri_np = np.triu(np.ones((128, 128), dtype=BF16NP))
    tri_d = nc.inline_tensor(tri_np, name="tri_tab").ap()
    ident_d = nc.inline_tensor(np.eye(128, dtype=BF16NP), name="ident_tab").ap()

    with tile.TileContext(nc) as tc:
        _body(tc, x_e, wq_e, wk_e, wv_e, wo_e, outp, kt_o, vt_o,
              cos_d, sin_d, tri_d, ident_d)

    nc.compile()
    return nc


def _body(tc, x_e, wq_e, wk_e, wv_e, wo_e, outp, kt_o, vt_o,
          cos_d, sin_d, tri_d, ident_d):
    nc = tc.nc

    import contextlib
    ctx = contextlib.ExitStack()
    with ctx:
        pe = ctx.enter_context(tc.tile_pool(name="persist", bufs=1))

        def ptile(shape, dtype, name):
            return pe.tile(shape, dtype, name=name, tag=name)

        # ---- constants into SBUF ----
        cos_t = ptile([128, T], BF16, "cos_t")
        sin_t = ptile([128, T], BF16, "sin_t")
        tri_t = ptile([128, 128], BF16, "tri_t")
        ident_t = ptile([128, 128], BF16, "ident_t")
        # ---- phase 1: xbar-transposed loads straight from bf16 inputs ----
        # Order matters: v weights first (v-projection runs first), then x,
        # then k/q weights, then wo.
        def wload(wbf, nm, eng):
            lst = []
            for c in range(CCH):
                t_ = ptile([128, HL], BF16, f"w{nm}T{c}")
                eng.dma_start(out=t_[:], in_=wbf[:, c * 128:(c + 1) * 128], transpose=True)
                lst.append(t_)
            return lst

        # v weights and x interleaved first (v-projection consumes them
        # first), then rope tables, then k/q/wo weights.
        wvT, xT = [], []
        for c in range(CCH):
            t_ = ptile([128, HL], BF16, f"wvT{c}")
            nc.sync.dma_start(out=t_[:], in_=wv_e[:, c * 128:(c + 1) * 128], transpose=True)
            wvT.append(t_)
            t_ = ptile([128, T], BF16, f"xT{c}")
            nc.sync.dma_start(out=t_[:], in_=x_e[:, c * 128:(c + 1) * 128], transpose=True)
            xT.append(t_)
        nc.sync.dma_start(out=cos_t[:], in_=cos_d[:])
        nc.sync.dma_start(out=sin_t[:], in_=sin_d[:])
        nc.sync.dma_start(out=tri_t[:], in_=tri_d[:])
        nc.sync.dma_start(out=ident_t[:], in_=ident_d[:])
        wkT = wload(wk_e, "k", nc.sync)
        wqT = wload(wq_e, "q", nc.sync)
        woT = []
        for m in range(MCH):
            t_ = ptile([128, C], BF16, f"woT{m}")
            nc.sync.dma_start(out=t_[:], in_=wo_e[:, m * 128:(m + 1) * 128], transpose=True)
            woT.append(t_)

        # ---- phases 2+3 interleaved: projections + rope + attention ----
        # v first (feeds the va pipeline), then per m-chunk: k/q projections
        # followed immediately by that chunk's two heads of attention, so the
        # ACT-bound attention of chunk m overlaps the PE-bound projections of
        # chunk m+1.
        qTs = [ptile([128, T], BF16, f"qTs{m}") for m in range(MCH)]
        kTs = [ptile([128, T], BF16, f"kTs{m}") for m in range(MCH)]
        va = [ptile([128, HPC, 80], BF16, f"va{t}") for t in range(TCH)]
        catT = [ptile([128, T], BF16, f"catT{m}") for m in range(MCH)]
        scale = float(1.0 / np.sqrt(np.float32(HD)))

        with tc.tile_pool(name="proj_psum", bufs=2, space="PSUM") as pp, \
             tc.tile_pool(name="s_psum", bufs=4, space="PSUM") as sp, \
             tc.tile_pool(name="o_psum", bufs=2, space="PSUM") as op, \
             tc.tile_pool(name="rope_tmp", bufs=6) as rp, \
             tc.tile_pool(name="e_pool", bufs=16) as epool, \
             tc.tile_pool(name="n_pool", bufs=4) as npool:

            def proj_round(wT, m, consume, ns):
                pss = []
                for i, n in enumerate(ns):
                    pool_ = pp if i < 2 else sp
                    tag_ = "fill" if i < 2 else "s_ps"
                    pss.append(pool_.tile([128, NT], F32, name=f"pp{n}", tag=tag_))
                for c in range(CCH):
                    for ps_, n in zip(pss, ns):
                        nc.tensor.matmul(
                            ps_[:],
                            wT[c][:, m * 128:(m + 1) * 128],
                            xT[c][:, n * NT:(n + 1) * NT],
                            start=(c == 0),
                            stop=(c == CCH - 1),
                        )
                for ps_, n in zip(pss, ns):
                    consume(n, ps_)

            def proj_chunk(wT, m, consume, borrow=False):
                # borrow=True (only safe before attention starts) runs all
                # four n-tiles as concurrent accumulation chains using the
                # idle s_psum slots, so arriving xT chunks feed 4 matmuls
                if borrow:
                    proj_round(wT, m, consume, (0, 1, 2, 3))
                else:
                    proj_round(wT, m, consume, (0, 1))
                    proj_round(wT, m, consume, (2, 3))

            def rope_consume(dst, cast_eng=None):
                # dst = raw*cos + swap32(raw*sin2), where sin2 carries the
                # signs and swap32 exchanges 32-row halves within each
                # 64-row head group. The swap happens on the products'
                # OUTPUT APs (input operands stay partition-aligned, which
                # the walrus verifier requires for SBUF pairs).
                def f(n, ps_):
                    sl = slice(n * NT, (n + 1) * NT)
                    raw = rp.tile([128, NT], BF16, name="raw", tag="raw")
                    if cast_eng is nc.vector:
                        nc.vector.tensor_copy(raw[:], ps_[:])  # psum f32 -> bf16
                    else:
                        nc.scalar.copy(raw[:], ps_[:])
                    tco = rp.tile([128, NT], BF16, name="tco", tag="tco")
                    tsi = rp.tile([128, NT], BF16, name="tsi", tag="tsi")
                    nc.vector.tensor_mul(tco[:], raw[:], cos_t[:, sl])
                    for h0 in (0, 64):
                        lo = slice(h0, h0 + 32)
                        hi = slice(h0 + 32, h0 + 64)
                        nc.vector.tensor_mul(tsi[lo, :], raw[hi, :], sin_t[hi, sl])
                        nc.vector.tensor_mul(tsi[hi, :], raw[lo, :], sin_t[lo, sl])
                    nc.vector.tensor_add(dst[:, sl], tco[:], tsi[:])
                return f

            def copy_consume(dst):
                def f(n, ps_):
                    sl = slice(n * NT, (n + 1) * NT)
                    if n % 2 == 0:
                        nc.vector.tensor_copy(dst[:, sl], ps_[:])
                    else:
                        nc.scalar.copy(dst[:, sl], ps_[:])
                return f

            def attention_quarter(h, j):
                m = h // 2
                p0 = 64 * (h % 2)
                qq = j * NT
                kimax = min(TCH - 1, 4 * j + 3)
                o_ps = op.tile([65, NT], F32, name="o_ps", tag="o_ps")
                for ki in range(kimax + 1):
                    kd = ki * 128
                    pre = max(0, kd - qq)
                    s_ps = sp.tile([128, NT], F32, name="s_ps", tag="s_ps")
                    nc.tensor.matmul(
                        s_ps[:, pre:NT],
                        kTs[m][p0:p0 + 64, kd:kd + 128],
                        qTs[m][p0:p0 + 64, qq + pre:qq + NT],
                        start=True,
                        stop=True,
                    )
                    e_sb = epool.tile([128, NT], BF16, name="e_sb", tag="e_sb")
                    nc.scalar.activation(
                        e_sb[:, pre:NT],
                        s_ps[:, pre:NT],
                        mybir.ActivationFunctionType.Exp,
                        scale=scale,
                    )
                    if qq <= kd:
                        nc.vector.tensor_mul(
                            e_sb[:, pre:pre + 128], e_sb[:, pre:pre + 128], tri_t[:]
                        )
                    # columns below the causal prefix carry no contribution
                    # from this k-chunk: trim instead of streaming zeros
                    nc.tensor.matmul(
                        o_ps[:, pre:NT],
                        va[ki][:, h, 0:65],
                        e_sb[:, pre:NT],
                        start=(ki == 0),
                        stop=(ki == kimax),
                    )
                rc = npool.tile([1, NT], F32, name="rc", tag="rc")
                rb = npool.tile([64, NT], F32, name="rb", tag="rb")
                nc.vector.reciprocal(rc[:], o_ps[64:65, :])
                nc.gpsimd.partition_broadcast(rb[:], rc[:])
                nc.vector.tensor_mul(
                    catT[m][p0:p0 + 64, qq:qq + NT], o_ps[0:64, :], rb[:]
                )

            def outproj_quarter(j, osb, ts=None):
                for t in (ts if ts is not None else range(4 * j, 4 * j + 4)):
                    for nn in range(C // NT):
                        o2 = pp.tile([128, NT], F32, name="o2", tag="fill")
                        for m in range(MCH):
                            nc.tensor.matmul(
                                o2[:],
                                catT[m][:, t * 128:(t + 1) * 128],
                                woT[m][:, nn * NT:(nn + 1) * NT],
                                start=(m == 0),
                                stop=(m == MCH - 1),
                            )
                        ot = osb.tile([128, NT], BF16, name="ot", tag="ot")
                        nc.vector.tensor_copy(ot[:], o2[:])
                        nc.sync.dma_start(
                            out=outp[t * 128:(t + 1) * 128, nn * NT:(nn + 1) * NT],
                            in_=ot[:],
                        )

            with tc.tile_pool(name="o_sb", bufs=8) as osb:
                def proj_v(trange):
                    # v projection straight into natural layout: stationary
                    # xT chunks, moving wvT. Runs before attention, so it
                    # borrows the idle s_psum slots to keep 6 accumulation
                    # chains in flight while x is still streaming in.
                    trange = list(trange)
                    for g0 in range(0, len(trange), 6):
                        grp = trange[g0:g0 + 6]
                        pvs = []
                        for i, t in enumerate(grp):
                            pool_ = pp if i < 2 else sp
                            tag_ = "fill" if i < 2 else "s_ps"
                            pvs.append(pool_.tile([128, HL], F32, name="pv", tag=tag_))
                        for c in range(CCH):
                            for pv, t in zip(pvs, grp):
                                nc.tensor.matmul(
                                    pv[:],
                                    xT[c][:, t * 128:(t + 1) * 128],
                                    wvT[c][:],
                                    start=(c == 0),
                                    stop=(c == CCH - 1),
                                )
                        for pv, t in zip(pvs, grp):
                            nc.vector.tensor_copy(
                                va[t][:, :, 0:64],
                                pv.rearrange("p (h d) -> p h d", h=HPC),
                            )
                            nc.gpsimd.memset(va[t][:, :, 64:65], 1.0)
                            nc.sync.dma_start(
                                out=vt_o[t * 128:(t + 1) * 128, :],
                                in_=va[t][:, :, 0:64],
                            )

                proj_v(range(TCH))
                # m=0: k/q projections then heads 0,1 (their ACT-bound
                # attention overlaps m=1's PE-bound projections below)
                kc0, qc0 = rope_consume(kTs[0]), rope_consume(qTs[0])
                proj_round(wkT, 0, kc0, (0, 1))
                proj_round(wqT, 0, qc0, (0, 1))
                proj_round(wkT, 0, kc0, (2, 3))
                proj_round(wqT, 0, qc0, (2, 3))
                nc.sync.dma_start(out=kt_o[0:128, :], in_=kTs[0][:])
                for j in range(T // NT):
                    attention_quarter(0, j)
                    attention_quarter(1, j)
                # m=1 projections + heads 2,3; the out-projection trails one
                # quarter behind as PE filler. The m=1 psum casts go to DVE
                # to keep ACT free for exp.
                kc1 = rope_consume(kTs[1], cast_eng=nc.vector)
                qc1 = rope_consume(qTs[1], cast_eng=nc.vector)
                proj_round(wkT, 1, kc1, (0, 1))
                proj_round(wqT, 1, qc1, (0, 1))
                proj_round(wkT, 1, kc1, (2, 3))
                proj_round(wqT, 1, qc1, (2, 3))
                nc.sync.dma_start(out=kt_o[128:256, :], in_=kTs[1][:])
                for j in range(T // NT):
                    attention_quarter(2, j)
                    if j > 0:
                        outproj_quarter(j - 1, osb, ts=range(4 * j - 4, 4 * j - 2))
                    attention_quarter(3, j)
                    if j > 0:
                        outproj_quarter(j - 1, osb, ts=range(4 * j - 2, 4 * j))
                outproj_quarter(3, osb)


def _get_nc():
    if "nc" not in _CACHE:
        _CACHE["nc"] = _build()
    return _CACHE["nc"]


def kernel(x, wq, wk, wv, wo, _trace=False, _trace_kwargs=None):
    nc = _get_nc()
    x = np.asarray(x, dtype=np.float32).astype(BF16NP)
    wq = np.asarray(wq, dtype=np.float32).astype(BF16NP)
    wk = np.asarray(wk, dtype=np.float32).astype(BF16NP)
    wv = np.asarray(wv, dtype=np.float32).astype(BF16NP)
    wo = np.asarray(wo, dtype=np.float32).astype(BF16NP)

    in_maps = []
    for core in range(NCORES):
        b = core // CPB
        g = core % CPB
        r = slice(g * HL, (g + 1) * HL)
        in_maps.append({
            "x": np.ascontiguousarray(x[b]),
            "wq": np.ascontiguousarray(wq[r, :]),
            "wk": np.ascontiguousarray(wk[r, :]),
            "wv": np.ascontiguousarray(wv[r, :]),
            "wo": np.ascontiguousarray(wo[:, r]),
        })

    res = run_bass_kernel_spmd(
        nc, in_maps, core_ids=list(range(NCORES)),
        trace=_trace, **(_trace_kwargs or {}),
    )
    _CACHE["last_res"] = res
    results = res.results

    out = np.zeros((B, T, C), dtype=np.float32)
    k = np.zeros((B, H, T, HD), dtype=np.float32)
    v = np.zeros((B, H, T, HD), dtype=np.float32)
    for core in range(NCORES):
        b = core // CPB
        g = core % CPB
        out[b] += np.asarray(results[core]["outp"], dtype=np.float32)
        kt = np.asarray(results[core]["kt"], dtype=np.float32)
        vo = np.asarray(results[core]["vo"], dtype=np.float32)
        for hl in range(HPC):
            k[b, g * HPC + hl] = kt[hl * HD:(hl + 1) * HD, :].T
            v[b, g * HPC + hl] = vo[:, hl * HD:(hl + 1) * HD]
    return out, k, v


# revision 83
# speedup vs baseline: 1.0009x; 1.0009x over previous
"""Distributed multi-head attention kernel for 8 TRN2 NeuronCores.

Sharding: data parallel over batch (B=2 -> 2 groups of 4 cores), tensor
parallel over heads within each group (16 heads -> 4 heads/core).
Each core computes q/k/v projections for its 4 heads, rope, causal
attention, and a partial out-projection through its 256 columns of wo.
The host casts inputs to bf16, sums the 4 partial outputs per batch in
f32, and assembles k/v.

All matmuls run in bf16 (f32 PSUM accumulation). Attention is computed
in "transposed" layout (channels on partitions, time on the free dim)
so that no on-chip transposes of the attention matrix are needed:
    S.T[k,q]  = kT.T @ qT          (lhsT=kT chunk, rhs=qT)
    outT[d,q] = [v|1].T @ expS.T   (lhsT=v natural + ones col, rhs=expS.T)
The ones column accumulates the softmax denominator for free.
"""

import numpy as np
import ml_dtypes

import concourse.bass as bass
import concourse.tile as tile
from concourse import bacc, mybir
from concourse.bass_utils import run_bass_kernel_spmd

F32 = mybir.dt.float32
BF16 = mybir.dt.bfloat16
BF16NP = ml_dtypes.bfloat16

B, T, C = 2, 2048, 1024
H, HD = 16, 64
NCORES = 8
CPB = NCORES // B        # cores per batch = 4
HPC = H // CPB           # heads per core = 4
HL = HPC * HD            # local channels = 256
MCH = HL // 128          # 128-partition chunks of local channels = 2
CCH = C // 128           # 128-chunks of model dim = 8
TCH = T // 128           # 128-chunks of time = 16
NT = 512                 # PSUM bank tile (f32)
ST = 1024                # attention strip width

_CACHE = {}


def _rope_tables():
    # Matches reference _rope (f32 math), then bf16 for 4x DVE ops.
    # Replicated to 128 partitions so every 32-row slice is aligned.
    inv_freq = (1.0 / (10000.0 ** (np.arange(0, HD, 2, dtype=np.float32) / HD))).astype(np.float32)
    pos = np.arange(T, dtype=np.float32)
    freqs = pos[None, :] * inv_freq[:, None]          # [32, T]
    cos = np.tile(np.cos(freqs), (4, 1)).astype(BF16NP)
    # Sign-folded sin table for the pre-swapped product trick:
    # rows [+sin, -sin, +sin, -sin] by 32-row groups.
    s = np.sin(freqs)
    sin2 = np.concatenate([s, -s, s, -s], axis=0).astype(BF16NP)
    return cos, sin2


def _build():
    nc = bacc.Bacc(
        "TRN2",
        target_bir_lowering=False,
        debug=False,
        enable_asserts=False,
        num_devices=NCORES,
    )

    x_e = nc.dram_tensor("x", [T, C], BF16, kind="ExternalInput").ap()
    wq_e = nc.dram_tensor("wq", [HL, C], BF16, kind="ExternalInput").ap()
    wk_e = nc.dram_tensor("wk", [HL, C], BF16, kind="ExternalInput").ap()
    wv_e = nc.dram_tensor("wv", [HL, C], BF16, kind="ExternalInput").ap()
    wo_e = nc.dram_tensor("wo", [C, HL], BF16, kind="ExternalInput").ap()

    outp = nc.dram_tensor("outp", [T, C], BF16, kind="ExternalOutput").ap()
    kt_o = nc.dram_tensor("kt", [HL, T], BF16, kind="ExternalOutput").ap()
    vt_o = nc.dram_tensor("vo", [T, HL], BF16, kind="ExternalOutput").ap()

    cos_np, sin_np = _rope_tables()
    cos_d = nc.inline_tensor(cos_np, name="cos_tab").ap()
    sin_d = nc.inline_tensor(sin_np, name="sin_tab").ap()
    # Causal keep-mask in (k, q) layout: keep where k <= q.
    tri_np = np.triu(np.ones((128, 128), dtype=BF16NP))
    tri_d = nc.inline_tensor(tri_np, name="tri_tab").ap()
    ident_d = nc.inline_tensor(np.eye(128, dtype=BF16NP), name="ident_tab").ap()

    with tile.TileContext(nc) as tc:
        _body(tc, x_e, wq_e, wk_e, wv_e, wo_e, outp, kt_o, vt_o,
              cos_d, sin_d, tri_d, ident_d)

    nc.compile()
    return nc


def _body(tc, x_e, wq_e, wk_e, wv_e, wo_e, outp, kt_o, vt_o,
          cos_d, sin_d, tri_d, ident_d):
    nc = tc.nc

    import contextlib
    ctx = contextlib.ExitStack()
    with ctx:
        pe = ctx.enter_context(tc.tile_pool(name="persist", bufs=1))

        def ptile(shape, dtype, name):
            return pe.tile(shape, dtype, name=name, tag=name)

        # ---- constants into SBUF ----
        cos_t = ptile([128, T], BF16, "cos_t")
        sin_t = ptile([128, T], BF16, "sin_t")
        tri_t = ptile([128, 128], BF16, "tri_t")
        # ---- phase 1: xbar-transposed loads straight from bf16 inputs ----
        # Order matters: v weights first (v-projection runs first), then x,
        # then k/q weights, then wo.
        def wload(wbf, nm, eng):
            lst = []
            for c in range(CCH):
                t_ = ptile([128, HL], BF16, f"w{nm}T{c}")
                eng.dma_start(out=t_[:], in_=wbf[:, c * 128:(c + 1) * 128], transpose=True)
                lst.append(t_)
            return lst

        # v weights and x interleaved first (v-projection consumes them
        # first), then rope tables, then k/q/wo weights.
        wvT, xT = [], []
        for c in range(CCH):
            t_ = ptile([128, HL], BF16, f"wvT{c}")
            nc.sync.dma_start(out=t_[:], in_=wv_e[:, c * 128:(c + 1) * 128], transpose=True)
            wvT.append(t_)
            t_ = ptile([128, T], BF16, f"xT{c}")
            nc.sync.dma_start(out=t_[:], in_=x_e[:, c * 128:(c + 1) * 128], transpose=True)
            xT.append(t_)
        nc.sync.dma_start(out=cos_t[:], in_=cos_d[:])
        nc.sync.dma_start(out=sin_t[:], in_=sin_d[:])
        nc.sync.dma_start(out=tri_t[:], in_=tri_d[:])
        wkT = wload(wk_e, "k", nc.sync)
        wqT = wload(wq_e, "q", nc.sync)
        woT = []
        for m in range(MCH):
            t_ = ptile([128, C], BF16, f"woT{m}")
            nc.sync.dma_start(out=t_[:], in_=wo_e[:, m * 128:(m + 1) * 128], transpose=True)
            woT.append(t_)

        # ---- phases 2+3 interleaved: projections + rope + attention ----
        # v first (feeds the va pipeline), then per m-chunk: k/q projections
        # followed immediately by that chunk's two heads of attention, so the
        # ACT-bound attention of chunk m overlaps the PE-bound projections of
        # chunk m+1.
        qTs = [ptile([128, T], BF16, f"qTs{m}") for m in range(MCH)]
        kTs = [ptile([128, T], BF16, f"kTs{m}") for m in range(MCH)]
        va = [ptile([128, HPC, 80], BF16, f"va{t}") for t in range(TCH)]
        catT = [ptile([128, T], BF16, f"catT{m}") for m in range(MCH)]
        scale = float(1.0 / np.sqrt(np.float32(HD)))

        with tc.tile_pool(name="proj_psum", bufs=2, space="PSUM") as pp, \
             tc.tile_pool(name="s_psum", bufs=4, space="PSUM") as sp, \
             tc.tile_pool(name="o_psum", bufs=2, space="PSUM") as op, \
             tc.tile_pool(name="rope_tmp", bufs=6) as rp, \
             tc.tile_pool(name="e_pool", bufs=16) as epool, \
             tc.tile_pool(name="n_pool", bufs=4) as npool:

            def proj_round(wT, m, consume, ns):
                pss = []
                for i, n in enumerate(ns):
                    pool_ = pp if i < 2 else sp
                    tag_ = "fill" if i < 2 else "s_ps"
                    pss.append(pool_.tile([128, NT], F32, name=f"pp{n}", tag=tag_))
                for c in range(CCH):
                    for ps_, n in zip(pss, ns):
                        nc.tensor.matmul(
                            ps_[:],
                            wT[c][:, m * 128:(m + 1) * 128],
                            xT[c][:, n * NT:(n + 1) * NT],
                            start=(c == 0),
                            stop=(c == CCH - 1),
                        )
                for ps_, n in zip(pss, ns):
                    consume(n, ps_)

            def proj_chunk(wT, m, consume, borrow=False):
                # borrow=True (only safe before attention starts) runs all
                # four n-tiles as concurrent accumulation chains using the
                # idle s_psum slots, so arriving xT chunks feed 4 matmuls
                if borrow:
                    proj_round(wT, m, consume, (0, 1, 2, 3))
                else:
                    proj_round(wT, m, consume, (0, 1))
                    proj_round(wT, m, consume, (2, 3))

            def rope_consume(dst, cast_eng=None):
                # dst = raw*cos + swap32(raw*sin2), where sin2 carries the
                # signs and swap32 exchanges 32-row halves within each
                # 64-row head group. The swap happens on the products'
                # OUTPUT APs (input operands stay partition-aligned, which
                # the walrus verifier requires for SBUF pairs).
                def f(n, ps_):
                    sl = slice(n * NT, (n + 1) * NT)
                    raw = rp.tile([128, NT], BF16, name="raw", tag="raw")
                    if cast_eng is nc.vector:
                        nc.vector.tensor_copy(raw[:], ps_[:])  # psum f32 -> bf16
                    else:
                        nc.scalar.copy(raw[:], ps_[:])
                    tco = rp.tile([128, NT], BF16, name="tco", tag="tco")
                    tsi = rp.tile([128, NT], BF16, name="tsi", tag="tsi")
                    nc.vector.tensor_mul(tco[:], raw[:], cos_t[:, sl])
                    for h0 in (0, 64):
                        lo = slice(h0, h0 + 32)
                        hi = slice(h0 + 32, h0 + 64)
                        nc.vector.tensor_mul(tsi[lo, :], raw[hi, :], sin_t[hi, sl])
                        nc.vector.tensor_mul(tsi[hi, :], raw[lo, :], sin_t[lo, sl])
                    nc.vector.tensor_add(dst[:, sl], tco[:], tsi[:])
                return f

            def copy_consume(dst):
                def f(n, ps_):
                    sl = slice(n * NT, (n + 1) * NT)
                    if n % 2 == 0:
                        nc.vector.tensor_copy(dst[:, sl], ps_[:])
                    else:
                        nc.scalar.copy(dst[:, sl], ps_[:])
                return f

            def attention_quarter(h, j):
                m = h // 2
                p0 = 64 * (h % 2)
                qq = j * NT
                kimax = min(TCH - 1, 4 * j + 3)
                o_ps = op.tile([65, NT], F32, name="o_ps", tag="o_ps")
                for ki in range(kimax + 1):
                    kd = ki * 128
                    pre = max(0, kd - qq)
                    s_ps = sp.tile([128, NT], F32, name="s_ps", tag="s_ps")
                    nc.tensor.matmul(
                        s_ps[:, pre:NT],
                        kTs[m][p0:p0 + 64, kd:kd + 128],
                        qTs[m][p0:p0 + 64, qq + pre:qq + NT],
                        start=True,
                        stop=True,
                    )
                    e_sb = epool.tile([128, NT], BF16, name="e_sb", tag="e_sb")
                    nc.scalar.activation(
                        e_sb[:, pre:NT],
                        s_ps[:, pre:NT],
                        mybir.ActivationFunctionType.Exp,
                        scale=scale,
                    )
                    if qq <= kd:
                        nc.vector.tensor_mul(
                            e_sb[:, pre:pre + 128], e_sb[:, pre:pre + 128], tri_t[:]
                        )
                    # columns below the causal prefix carry no contribution
                    # from this k-chunk: trim instead of streaming zeros
                    nc.tensor.matmul(
                        o_ps[:, pre:NT],
                        va[ki][:, h, 0:65],
                        e_sb[:, pre:NT],
                        start=(ki == 0),
                        stop=(ki == kimax),
                    )
                rc = npool.tile([1, NT], F32, name="rc", tag="rc")
                rb = npool.tile([64, NT], F32, name="rb", tag="rb")
                nc.vector.reciprocal(rc[:], o_ps[64:65, :])
                nc.gpsimd.partition_broadcast(rb[:], rc[:])
                nc.vector.tensor_mul(
                    catT[m][p0:p0 + 64, qq:qq + NT], o_ps[0:64, :], rb[:]
                )

            def outproj_quarter(j, osb, ts=None):
                for t in (ts if ts is not None else range(4 * j, 4 * j + 4)):
                    for nn in range(C // NT):
                        o2 = pp.tile([128, NT], F32, name="o2", tag="fill")
                        for m in range(MCH):
                            nc.tensor.matmul(
                                o2[:],
                                catT[m][:, t * 128:(t + 1) * 128],
                                woT[m][:, nn * NT:(nn + 1) * NT],
                                start=(m == 0),
                                stop=(m == MCH - 1),
                            )
                        ot = osb.tile([128, NT], BF16, name="ot", tag="ot")
                        nc.vector.tensor_copy(ot[:], o2[:])
                        nc.sync.dma_start(
                            out=outp[t * 128:(t + 1) * 128, nn * NT:(nn + 1) * NT],
                            in_=ot[:],
                        )

            with tc.tile_pool(name="o_sb", bufs=8) as osb:
                def proj_v(trange):
                    # v projection straight into natural layout: stationary
                    # xT chunks, moving wvT. Runs before attention, so it
                    # borrows the idle s_psum slots to keep 6 accumulation
                    # chains in flight while x is still streaming in.
                    trange = list(trange)
                    for g0 in range(0, len(trange), 6):
                        grp = trange[g0:g0 + 6]
                        pvs = []
                        for i, t in enumerate(grp):
                            pool_ = pp if i < 2 else sp
                            tag_ = "fill" if i < 2 else "s_ps"
                            pvs.append(pool_.tile([128, HL], F32, name="pv", tag=tag_))
                        for c in range(CCH):
                            for pv, t in zip(pvs, grp):
                                nc.tensor.matmul(
                                    pv[:],
                                    xT[c][:, t * 128:(t + 1) * 128],
                                    wvT[c][:],
                                    start=(c == 0),
                                    stop=(c == CCH - 1),
                                )
                        for pv, t in zip(pvs, grp):
                            nc.vector.tensor_copy(
                                va[t][:, :, 0:64],
                                pv.rearrange("p (h d) -> p h d", h=HPC),
                            )
                            nc.gpsimd.memset(va[t][:, :, 64:65], 1.0)
                            nc.sync.dma_start(
                                out=vt_o[t * 128:(t + 1) * 128, :],
                                in_=va[t][:, :, 0:64],
                            )

                proj_v(range(TCH))
                # m=0: k/q projections then heads 0,1 (their ACT-bound
                # attention overlaps m=1's PE-bound projections below)
                kc0, qc0 = rope_consume(kTs[0]), rope_consume(qTs[0])
                proj_round(wkT, 0, kc0, (0, 1))
                proj_round(wqT, 0, qc0, (0, 1))
                proj_round(wkT, 0, kc0, (2, 3))
                proj_round(wqT, 0, qc0, (2, 3))
                nc.sync.dma_start(out=kt_o[0:128, :], in_=kTs[0][:])
                for j in range(T // NT):
                    attention_quarter(0, j)
                    attention_quarter(1, j)
                # m=1 projections + heads 2,3; the out-projection trails one
                # quarter behind as PE filler. The m=1 psum casts go to DVE
                # to keep ACT free for exp.
                kc1 = rope_consume(kTs[1], cast_eng=nc.vector)
                qc1 = rope_consume(qTs[1], cast_eng=nc.vector)
                proj_round(wkT, 1, kc1, (0, 1))
                proj_round(wqT, 1, qc1, (0, 1))
                proj_round(wkT, 1, kc1, (2, 3))
                proj_round(wqT, 1, qc1, (2, 3))
                nc.sync.dma_start(out=kt_o[128:256, :], in_=kTs[1][:])
                for j in range(T // NT):
                    attention_quarter(2, j)
                    if j > 0:
                        outproj_quarter(j - 1, osb, ts=range(4 * j - 4, 4 * j - 2))
                    attention_quarter(3, j)
                    if j > 0:
                        outproj_quarter(j - 1, osb, ts=range(4 * j - 2, 4 * j))
                outproj_quarter(3, osb)


def _get_nc():
    if "nc" not in _CACHE:
        _CACHE["nc"] = _build()
    return _CACHE["nc"]


def kernel(x, wq, wk, wv, wo, _trace=False, _trace_kwargs=None):
    nc = _get_nc()
    x = np.asarray(x, dtype=np.float32).astype(BF16NP)
    wq = np.asarray(wq, dtype=np.float32).astype(BF16NP)
    wk = np.asarray(wk, dtype=np.float32).astype(BF16NP)
    wv = np.asarray(wv, dtype=np.float32).astype(BF16NP)
    wo = np.asarray(wo, dtype=np.float32).astype(BF16NP)

    in_maps = []
    for core in range(NCORES):
        b = core // CPB
        g = core % CPB
        r = slice(g * HL, (g + 1) * HL)
        in_maps.append({
            "x": np.ascontiguousarray(x[b]),
            "wq": np.ascontiguousarray(wq[r, :]),
            "wk": np.ascontiguousarray(wk[r, :]),
            "wv": np.ascontiguousarray(wv[r, :]),
            "wo": np.ascontiguousarray(wo[:, r]),
        })

    res = run_bass_kernel_spmd(
        nc, in_maps, core_ids=list(range(NCORES)),
        trace=_trace, **(_trace_kwargs or {}),
    )
    _CACHE["last_res"] = res
    results = res.results

    out = np.zeros((B, T, C), dtype=np.float32)
    k = np.zeros((B, H, T, HD), dtype=np.float32)
    v = np.zeros((B, H, T, HD), dtype=np.float32)
    for core in range(NCORES):
        b = core // CPB
        g = core % CPB
        out[b] += np.asarray(results[core]["outp"], dtype=np.float32)
        kt = np.asarray(results[core]["kt"], dtype=np.float32)
        vo = np.asarray(results[core]["vo"], dtype=np.float32)
        for hl in range(HPC):
            k[b, g * HPC + hl] = kt[hl * HD:(hl + 1) * HD, :].T
            v[b, g * HPC + hl] = vo[:, hl * HD:(hl + 1) * HD]
    return out, k, v


# revision 84
# speedup vs baseline: 1.0045x; 1.0037x over previous
"""Distributed multi-head attention kernel for 8 TRN2 NeuronCores.

Sharding: data parallel over batch (B=2 -> 2 groups of 4 cores), tensor
parallel over heads within each group (16 heads -> 4 heads/core).
Each core computes q/k/v projections for its 4 heads, rope, causal
attention, and a partial out-projection through its 256 columns of wo.
The host casts inputs to bf16, sums the 4 partial outputs per batch in
f32, and assembles k/v.

All matmuls run in bf16 (f32 PSUM accumulation). Attention is computed
in "transposed" layout (channels on partitions, time on the free dim)
so that no on-chip transposes of the attention matrix are needed:
    S.T[k,q]  = kT.T @ qT          (lhsT=kT chunk, rhs=qT)
    outT[d,q] = [v|1].T @ expS.T   (lhsT=v natural + ones col, rhs=expS.T)
The ones column accumulates the softmax denominator for free.
"""

import numpy as np
import ml_dtypes

import concourse.bass as bass
import concourse.tile as tile
from concourse import bacc, mybir
from concourse.bass_utils import run_bass_kernel_spmd

F32 = mybir.dt.float32
BF16 = mybir.dt.bfloat16
BF16NP = ml_dtypes.bfloat16

B, T, C = 2, 2048, 1024
H, HD = 16, 64
NCORES = 8
CPB = NCORES // B        # cores per batch = 4
HPC = H // CPB           # heads per core = 4
HL = HPC * HD            # local channels = 256
MCH = HL // 128          # 128-partition chunks of local channels = 2
CCH = C // 128           # 128-chunks of model dim = 8
TCH = T // 128           # 128-chunks of time = 16
NT = 512                 # PSUM bank tile (f32)
ST = 1024                # attention strip width

_CACHE = {}


def _rope_tables():
    # Matches reference _rope (f32 math), then bf16 for 4x DVE ops.
    # Replicated to 128 partitions so every 32-row slice is aligned.
    inv_freq = (1.0 / (10000.0 ** (np.arange(0, HD, 2, dtype=np.float32) / HD))).astype(np.float32)
    pos = np.arange(T, dtype=np.float32)
    freqs = pos[None, :] * inv_freq[:, None]          # [32, T]
    cos = np.tile(np.cos(freqs), (4, 1)).astype(BF16NP)
    # Sign-folded sin table for the pre-swapped product trick:
    # rows [+sin, -sin, +sin, -sin] by 32-row groups.
    s = np.sin(freqs)
    sin2 = np.concatenate([s, -s, s, -s], axis=0).astype(BF16NP)
    return cos, sin2


def _build():
    nc = bacc.Bacc(
        "TRN2",
        target_bir_lowering=False,
        debug=False,
        enable_asserts=False,
        num_devices=NCORES,
    )

    x_e = nc.dram_tensor("x", [T, C], BF16, kind="ExternalInput").ap()
    wq_e = nc.dram_tensor("wq", [HL, C], BF16, kind="ExternalInput").ap()
    wk_e = nc.dram_tensor("wk", [HL, C], BF16, kind="ExternalInput").ap()
    wv_e = nc.dram_tensor("wv", [HL, C], BF16, kind="ExternalInput").ap()
    wo_e = nc.dram_tensor("wo", [C, HL], BF16, kind="ExternalInput").ap()

    outp = nc.dram_tensor("outp", [T, C], BF16, kind="ExternalOutput").ap()
    kt_o = nc.dram_tensor("kt", [HL, T], BF16, kind="ExternalOutput").ap()
    vt_o = nc.dram_tensor("vo", [T, HL], BF16, kind="ExternalOutput").ap()

    cos_np, sin_np = _rope_tables()
    cos_d = nc.inline_tensor(cos_np, name="cos_tab").ap()
    sin_d = nc.inline_tensor(sin_np, name="sin_tab").ap()
    # Causal keep-mask in (k, q) layout: keep where k <= q.
    tri_np = np.triu(np.ones((128, 128), dtype=BF16NP))
    tri_d = nc.inline_tensor(tri_np, name="tri_tab").ap()
    ident_d = nc.inline_tensor(np.eye(128, dtype=BF16NP), name="ident_tab").ap()

    with tile.TileContext(nc) as tc:
        _body(tc, x_e, wq_e, wk_e, wv_e, wo_e, outp, kt_o, vt_o,
              cos_d, sin_d, tri_d, ident_d)

    nc.compile()
    return nc


def _body(tc, x_e, wq_e, wk_e, wv_e, wo_e, outp, kt_o, vt_o,
          cos_d, sin_d, tri_d, ident_d):
    nc = tc.nc

    import contextlib
    ctx = contextlib.ExitStack()
    with ctx:
        pe = ctx.enter_context(tc.tile_pool(name="persist", bufs=1))

        def ptile(shape, dtype, name):
            return pe.tile(shape, dtype, name=name, tag=name)

        # ---- constants into SBUF ----
        cos_t = ptile([128, T], BF16, "cos_t")
        sin_t = ptile([128, T], BF16, "sin_t")
        tri_t = ptile([128, 128], BF16, "tri_t")
        # ---- phase 1: xbar-transposed loads straight from bf16 inputs ----
        # Order matters: v weights first (v-projection runs first), then x,
        # then k/q weights, then wo.
        def wload(wbf, nm, eng):
            lst = []
            for c in range(CCH):
                t_ = ptile([128, HL], BF16, f"w{nm}T{c}")
                eng.dma_start(out=t_[:], in_=wbf[:, c * 128:(c + 1) * 128], transpose=True)
                lst.append(t_)
            return lst

        # v weights and x interleaved first (v-projection consumes them
        # first), then rope tables, then k/q/wo weights.
        wvT, xT = [], []
        for c in range(CCH):
            t_ = ptile([128, HL], BF16, f"wvT{c}")
            nc.sync.dma_start(out=t_[:], in_=wv_e[:, c * 128:(c + 1) * 128], transpose=True)
            wvT.append(t_)
            t_ = ptile([128, T], BF16, f"xT{c}")
            nc.sync.dma_start(out=t_[:], in_=x_e[:, c * 128:(c + 1) * 128], transpose=True)
            xT.append(t_)
        wkT = wload(wk_e, "k", nc.sync)
        wqT = wload(wq_e, "q", nc.sync)
        nc.sync.dma_start(out=cos_t[:], in_=cos_d[:])
        nc.sync.dma_start(out=sin_t[:], in_=sin_d[:])
        nc.sync.dma_start(out=tri_t[:], in_=tri_d[:])
        woT = []
        for m in range(MCH):
            t_ = ptile([128, C], BF16, f"woT{m}")
            nc.sync.dma_start(out=t_[:], in_=wo_e[:, m * 128:(m + 1) * 128], transpose=True)
            woT.append(t_)

        # ---- phases 2+3 interleaved: projections + rope + attention ----
        # v first (feeds the va pipeline), then per m-chunk: k/q projections
        # followed immediately by that chunk's two heads of attention, so the
        # ACT-bound attention of chunk m overlaps the PE-bound projections of
        # chunk m+1.
        qTs = [ptile([128, T], BF16, f"qTs{m}") for m in range(MCH)]
        kTs = [ptile([128, T], BF16, f"kTs{m}") for m in range(MCH)]
        va = [ptile([128, HPC, 80], BF16, f"va{t}") for t in range(TCH)]
        catT = [ptile([128, T], BF16, f"catT{m}") for m in range(MCH)]
        scale = float(1.0 / np.sqrt(np.float32(HD)))

        with tc.tile_pool(name="proj_psum", bufs=2, space="PSUM") as pp, \
             tc.tile_pool(name="s_psum", bufs=4, space="PSUM") as sp, \
             tc.tile_pool(name="o_psum", bufs=2, space="PSUM") as op, \
             tc.tile_pool(name="rope_tmp", bufs=6) as rp, \
             tc.tile_pool(name="e_pool", bufs=16) as epool, \
             tc.tile_pool(name="n_pool", bufs=4) as npool:

            def proj_round(wT, m, consume, ns):
                pss = []
                for i, n in enumerate(ns):
                    pool_ = pp if i < 2 else sp
                    tag_ = "fill" if i < 2 else "s_ps"
                    pss.append(pool_.tile([128, NT], F32, name=f"pp{n}", tag=tag_))
                for c in range(CCH):
                    for ps_, n in zip(pss, ns):
                        nc.tensor.matmul(
                            ps_[:],
                            wT[c][:, m * 128:(m + 1) * 128],
                            xT[c][:, n * NT:(n + 1) * NT],
                            start=(c == 0),
                            stop=(c == CCH - 1),
                        )
                for ps_, n in zip(pss, ns):
                    consume(n, ps_)

            def proj_chunk(wT, m, consume, borrow=False):
                # borrow=True (only safe before attention starts) runs all
                # four n-tiles as concurrent accumulation chains using the
                # idle s_psum slots, so arriving xT chunks feed 4 matmuls
                if borrow:
                    proj_round(wT, m, consume, (0, 1, 2, 3))
                else:
                    proj_round(wT, m, consume, (0, 1))
                    proj_round(wT, m, consume, (2, 3))

            def rope_consume(dst, cast_eng=None):
                # dst = raw*cos + swap32(raw*sin2), where sin2 carries the
                # signs and swap32 exchanges 32-row halves within each
                # 64-row head group. The swap happens on the products'
                # OUTPUT APs (input operands stay partition-aligned, which
                # the walrus verifier requires for SBUF pairs).
                def f(n, ps_):
                    sl = slice(n * NT, (n + 1) * NT)
                    raw = rp.tile([128, NT], BF16, name="raw", tag="raw")
                    if cast_eng is nc.vector:
                        nc.vector.tensor_copy(raw[:], ps_[:])  # psum f32 -> bf16
                    else:
                        nc.scalar.copy(raw[:], ps_[:])
                    tco = rp.tile([128, NT], BF16, name="tco", tag="tco")
                    tsi = rp.tile([128, NT], BF16, name="tsi", tag="tsi")
                    nc.vector.tensor_mul(tco[:], raw[:], cos_t[:, sl])
                    for h0 in (0, 64):
                        lo = slice(h0, h0 + 32)
                        hi = slice(h0 + 32, h0 + 64)
                        nc.vector.tensor_mul(tsi[lo, :], raw[hi, :], sin_t[hi, sl])
                        nc.vector.tensor_mul(tsi[hi, :], raw[lo, :], sin_t[lo, sl])
                    nc.vector.tensor_add(dst[:, sl], tco[:], tsi[:])
                return f

            def copy_consume(dst):
                def f(n, ps_):
                    sl = slice(n * NT, (n + 1) * NT)
                    if n % 2 == 0:
                        nc.vector.tensor_copy(dst[:, sl], ps_[:])
                    else:
                        nc.scalar.copy(dst[:, sl], ps_[:])
                return f

            def attention_quarter(h, j):
                m = h // 2
                p0 = 64 * (h % 2)
                qq = j * NT
                kimax = min(TCH - 1, 4 * j + 3)
                o_ps = op.tile([65, NT], F32, name="o_ps", tag="o_ps")
                for ki in range(kimax + 1):
                    kd = ki * 128
                    pre = max(0, kd - qq)
                    s_ps = sp.tile([128, NT], F32, name="s_ps", tag="s_ps")
                    nc.tensor.matmul(
                        s_ps[:, pre:NT],
                        kTs[m][p0:p0 + 64, kd:kd + 128],
                        qTs[m][p0:p0 + 64, qq + pre:qq + NT],
                        start=True,
                        stop=True,
                    )
                    e_sb = epool.tile([128, NT], BF16, name="e_sb", tag="e_sb")
                    nc.scalar.activation(
                        e_sb[:, pre:NT],
                        s_ps[:, pre:NT],
                        mybir.ActivationFunctionType.Exp,
                        scale=scale,
                    )
                    if qq <= kd:
                        nc.vector.tensor_mul(
                            e_sb[:, pre:pre + 128], e_sb[:, pre:pre + 128], tri_t[:]
                        )
                    # columns below the causal prefix carry no contribution
                    # from this k-chunk: trim instead of streaming zeros
                    nc.tensor.matmul(
                        o_ps[:, pre:NT],
                        va[ki][:, h, 0:65],
                        e_sb[:, pre:NT],
                        start=(ki == 0),
                        stop=(ki == kimax),
                    )
                rc = npool.tile([1, NT], F32, name="rc", tag="rc")
                rb = npool.tile([64, NT], F32, name="rb", tag="rb")
                nc.vector.reciprocal(rc[:], o_ps[64:65, :])
                nc.gpsimd.partition_broadcast(rb[:], rc[:])
                nc.vector.tensor_mul(
                    catT[m][p0:p0 + 64, qq:qq + NT], o_ps[0:64, :], rb[:]
                )

            def outproj_quarter(j, osb, ts=None):
                for t in (ts if ts is not None else range(4 * j, 4 * j + 4)):
                    for nn in range(C // NT):
                        o2 = pp.tile([128, NT], F32, name="o2", tag="fill")
                        for m in range(MCH):
                            nc.tensor.matmul(
                                o2[:],
                                catT[m][:, t * 128:(t + 1) * 128],
                                woT[m][:, nn * NT:(nn + 1) * NT],
                                start=(m == 0),
                                stop=(m == MCH - 1),
                            )
                        ot = osb.tile([128, NT], BF16, name="ot", tag="ot")
                        nc.vector.tensor_copy(ot[:], o2[:])
                        nc.sync.dma_start(
                            out=outp[t * 128:(t + 1) * 128, nn * NT:(nn + 1) * NT],
                            in_=ot[:],
                        )

            with tc.tile_pool(name="o_sb", bufs=8) as osb:
                def proj_v(trange):
                    # v projection straight into natural layout: stationary
                    # xT chunks, moving wvT. Runs before attention, so it
                    # borrows the idle s_psum slots to keep 6 accumulation
                    # chains in flight while x is still streaming in.
                    trange = list(trange)
                    for g0 in range(0, len(trange), 6):
                        grp = trange[g0:g0 + 6]
                        pvs = []
                        for i, t in enumerate(grp):
                            pool_ = pp if i < 2 else sp
                            tag_ = "fill" if i < 2 else "s_ps"
                            pvs.append(pool_.tile([128, HL], F32, name="pv", tag=tag_))
                        for c in range(CCH):
                            for pv, t in zip(pvs, grp):
                                nc.tensor.matmul(
                                    pv[:],
                                    xT[c][:, t * 128:(t + 1) * 128],
                                    wvT[c][:],
                                    start=(c == 0),
                                    stop=(c == CCH - 1),
                                )
                        for pv, t in zip(pvs, grp):
                            nc.vector.tensor_copy(
                                va[t][:, :, 0:64],
                                pv.rearrange("p (h d) -> p h d", h=HPC),
                            )
                            nc.gpsimd.memset(va[t][:, :, 64:65], 1.0)
                            nc.sync.dma_start(
                                out=vt_o[t * 128:(t + 1) * 128, :],
                                in_=va[t][:, :, 0:64],
                            )

                proj_v(range(TCH))
                # m=0: k/q projections then heads 0,1 (their ACT-bound
                # attention overlaps m=1's PE-bound projections below)
                kc0, qc0 = rope_consume(kTs[0]), rope_consume(qTs[0])
                proj_round(wkT, 0, kc0, (0, 1))
                proj_round(wqT, 0, qc0, (0, 1))
                proj_round(wkT, 0, kc0, (2, 3))
                proj_round(wqT, 0, qc0, (2, 3))
                nc.sync.dma_start(out=kt_o[0:128, :], in_=kTs[0][:])
                for j in range(T // NT):
                    attention_quarter(0, j)
                    attention_quarter(1, j)
                # m=1 projections + heads 2,3; the out-projection trails one
                # quarter behind as PE filler. The m=1 psum casts go to DVE
                # to keep ACT free for exp.
                kc1 = rope_consume(kTs[1], cast_eng=nc.vector)
                qc1 = rope_consume(qTs[1], cast_eng=nc.vector)
                proj_round(wkT, 1, kc1, (0, 1))
                proj_round(wqT, 1, qc1, (0, 1))
                proj_round(wkT, 1, kc1, (2, 3))
                proj_round(wqT, 1, qc1, (2, 3))
                nc.sync.dma_start(out=kt_o[128:256, :], in_=kTs[1][:])
                for j in range(T // NT):
                    attention_quarter(2, j)
                    if j > 0:
                        outproj_quarter(j - 1, osb, ts=range(4 * j - 4, 4 * j - 2))
                    attention_quarter(3, j)
                    if j > 0:
                        outproj_quarter(j - 1, osb, ts=range(4 * j - 2, 4 * j))
                outproj_quarter(3, osb)


def _get_nc():
    if "nc" not in _CACHE:
        _CACHE["nc"] = _build()
    return _CACHE["nc"]


def kernel(x, wq, wk, wv, wo, _trace=False, _trace_kwargs=None):
    nc = _get_nc()
    x = np.asarray(x, dtype=np.float32).astype(BF16NP)
    wq = np.asarray(wq, dtype=np.float32).astype(BF16NP)
    wk = np.asarray(wk, dtype=np.float32).astype(BF16NP)
    wv = np.asarray(wv, dtype=np.float32).astype(BF16NP)
    wo = np.asarray(wo, dtype=np.float32).astype(BF16NP)

    in_maps = []
    for core in range(NCORES):
        b = core // CPB
        g = core % CPB
        r = slice(g * HL, (g + 1) * HL)
        in_maps.append({
            "x": np.ascontiguousarray(x[b]),
            "wq": np.ascontiguousarray(wq[r, :]),
            "wk": np.ascontiguousarray(wk[r, :]),
            "wv": np.ascontiguousarray(wv[r, :]),
            "wo": np.ascontiguousarray(wo[:, r]),
        })

    res = run_bass_kernel_spmd(
        nc, in_maps, core_ids=list(range(NCORES)),
        trace=_trace, **(_trace_kwargs or {}),
    )
    _CACHE["last_res"] = res
    results = res.results

    out = np.zeros((B, T, C), dtype=np.float32)
    k = np.zeros((B, H, T, HD), dtype=np.float32)
    v = np.zeros((B, H, T, HD), dtype=np.float32)
    for core in range(NCORES):
        b = core // CPB
        g = core % CPB
        out[b] += np.asarray(results[core]["outp"], dtype=np.float32)
        kt = np.asarray(results[core]["kt"], dtype=np.float32)
        vo = np.asarray(results[core]["vo"], dtype=np.float32)
        for hl in range(HPC):
            k[b, g * HPC + hl] = kt[hl * HD:(hl + 1) * HD, :].T
            v[b, g * HPC + hl] = vo[:, hl * HD:(hl + 1) * HD]
    return out, k, v
